# revision 38
# baseline (speedup 1.0000x reference)
"""MoE FFN (8 experts, top-2, SwiGLU) Trainium2 kernel — bf16 quarter-expert.

Sharding: each expert's hidden dim H=2048 is split into four quarters;
the 32 (expert, H-quarter) jobs are packed 4-per-core in 4 SPMD slots.
Slot s holds the quarters of the (2s, 2s+1)-heaviest-loaded experts
(per the host router), so the per-slot token capacity tracks the actual
expert loads (1092/1028/1020/1000 here vs 1152 for plain
expert-parallel SPMD). The four quarters of an expert produce partial
y sums the host adds during scatter.

The router (top-2 + combine weights) runs on host, exactly replicating
the reference. FFN matmuls are bf16 (measured PE rate: 1 cycle per
moving row, weight loads fully hidden). Phase A computes hT[h,tok]
h-major (g-pass then v-pass sharing PSUM tags generationally); phase B
computes yT[d,tok] d-major (wo stationary, exact token counts — no
padded-token rows); y is stored bf16 and combine-scaled on host.

Self-contained: shapes/sharding hardcoded for
x[2,2048,1024], 8 experts, d_expert=2048, top-2.
"""

import math
from contextlib import ExitStack

import ml_dtypes
import numpy as np

import concourse.mybir as mybir
import concourse.tile as tile
from concourse import bacc
from concourse.bass_utils import run_bass_kernel_spmd

# ---- problem constants --------------------------------------------------
B, T, D = 2, 2048, 1024
N_TOK = B * T          # 4096 tokens
E = 8                  # experts
H = 2048               # expert hidden dim
HQ = H // 4            # per-slot hidden quarter (512)
TOP_K = 2
P = 128
NS = 4                 # slots per core

CAPS = (1092, 1028, 1020, 1000)   # per-slot token capacity
ND = D // P            # 8  d-tiles
NHQ = HQ // P          # 4  h-tiles per slot

BFD = mybir.dt.bfloat16
FP = mybir.dt.float32
AF = mybir.ActivationFunctionType
OP = mybir.AluOpType
BF = ml_dtypes.bfloat16


def _chunks(cap):
    out, cs = [], 0
    while cs < cap:
        cw = min(512, cap - cs)
        out.append((cs, cw))
        cs += cw
    return out


CHUNKS = tuple(_chunks(c) for c in CAPS)


def _emit(nc, tc, ctx, tens):
    const = ctx.enter_context(tc.tile_pool(name="const", bufs=1))
    wsb = ctx.enter_context(tc.tile_pool(name="wsb", bufs=1))
    wgv = ctx.enter_context(tc.tile_pool(name="wgv", bufs=2))
    htp = ctx.enter_context(tc.tile_pool(name="htp", bufs=1))
    act = ctx.enter_context(tc.tile_pool(name="act", bufs=2))
    yst = ctx.enter_context(tc.tile_pool(name="yst", bufs=2))

    x_sb, x_ap, wg_ap, wv_ap, wo_ap = [], [], [], [], []
    wo_sb = []
    for s in range(NS):
        x_sb.append(const.tile([P, ND, CAPS[s]], BFD, name=f"x{s}"))
        x_ap.append(tens[f"x{s}"].ap().rearrange("(j p) c -> p j c", p=P))
        wg_ap.append(tens[f"wg{s}"].ap().rearrange("(j p) h -> p j h", p=P))
        wv_ap.append(tens[f"wv{s}"].ap().rearrange("(j p) h -> p j h", p=P))
        wo_sb.append(wsb.tile([P, NHQ, D], BFD, name=f"wo{s}"))
        wo_ap.append(tens[f"wo{s}"].ap().rearrange("(j p) d -> p j d", p=P))

    # wg/wv stream through a 2-deep pool (dead after their slot's A pass)
    wg_t, wv_t = [None] * NS, [None] * NS
    for s in range(NS):
        wg_t[s] = wgv.tile([P, ND, HQ], BFD, name=f"wg{s}", tag="wg")
        wv_t[s] = wgv.tile([P, ND, HQ], BFD, name=f"wv{s}", tag="wv")

    # head-latency-ordered loads on the sync queue, first-use order;
    # the scalar queue configures slot0's v-pass head load in parallel.
    nc.sync.dma_start(out=x_sb[0][:, 0:1, 0:512], in_=x_ap[0][:, 0:1, 0:512])
    nc.sync.dma_start(out=wg_t[0][:, :, 0:P], in_=wg_ap[0][:, :, 0:P])
    nc.sync.dma_start(out=x_sb[0][:, 0:1, 512:], in_=x_ap[0][:, 0:1, 512:])
    nc.sync.dma_start(out=x_sb[0][:, 1:2], in_=x_ap[0][:, 1:2])
    nc.sync.dma_start(out=x_sb[0][:, 2:4], in_=x_ap[0][:, 2:4])
    nc.sync.dma_start(out=x_sb[0][:, 4:6], in_=x_ap[0][:, 4:6])
    nc.sync.dma_start(out=x_sb[0][:, 6:8], in_=x_ap[0][:, 6:8])
    nc.scalar.dma_start(out=wv_t[0][:, :, 0:P], in_=wv_ap[0][:, :, 0:P])
    nc.sync.dma_start(out=wg_t[0][:, :, P:HQ], in_=wg_ap[0][:, :, P:HQ])
    nc.sync.dma_start(out=wv_t[0][:, :, P:HQ], in_=wv_ap[0][:, :, P:HQ])
    # slot1's wg/wv use the pool's second buffers (no wait); slots 2-3
    # reuse buffers freed by earlier slots' A passes, so their DMAs
    # carry waits — emit them LAST so they cannot head-of-line-block
    # the independent x/wo loads on the queue.
    nc.sync.dma_start(out=wg_t[1][:], in_=wg_ap[1])
    nc.sync.dma_start(out=wv_t[1][:], in_=wv_ap[1])
    for s in range(1, NS):
        nc.sync.dma_start(out=x_sb[s][:, 0:4], in_=x_ap[s][:, 0:4])
        nc.sync.dma_start(out=x_sb[s][:, 4:8], in_=x_ap[s][:, 4:8])
    for s in range(NS):
        nc.sync.dma_start(out=wo_sb[s][:], in_=wo_ap[s])
    for s in range(2, NS):
        nc.sync.dma_start(out=wg_t[s][:], in_=wg_ap[s])
        nc.sync.dma_start(out=wv_t[s][:], in_=wv_ap[s])

    ht = [[htp.tile([P, CAPS[s]], BFD, name=f"ht{s}_{k}") for k in range(NHQ)]
          for s in range(NS)]

    # PE p-state warm-up on a zeroed tile (no DMA dependency)
    warmz = const.tile([P, 2 * P], BFD)
    nc.vector.memset(warmz[:], 0.0)
    with ExitStack() as wctx:
        ps_w = wctx.enter_context(tc.tile_pool(name="psw", bufs=1, space="PSUM"))
        warm = ps_w.tile([P, 2 * P], FP, name="warm", tag="warm")
        for _ in range(24):
            nc.tensor.matmul(warm[:], lhsT=warmz[:, 0:P], rhs=warmz[:],
                             start=True, stop=True)

    # ---- phase A: hT[h, tok] = silu(x@wg)^T * (x@wv)^T ------------------
    with ExitStack() as actx:
        ps_a = actx.enter_context(tc.tile_pool(name="psa", bufs=3, space="PSUM"))
        for s in range(NS):
            for hk in range(NHQ):
                hs = slice(hk * P, (hk + 1) * P)
                pgs = [ps_a.tile([P, cw], FP, name=f"pg{ci}", tag=f"p{ci}",
                                 bufs=(2 if ci == 2 else 3))
                       for ci, (_, cw) in enumerate(CHUNKS[s])]
                pvs = [ps_a.tile([P, cw], FP, name=f"pv{ci}", tag=f"p{ci}",
                                 bufs=(2 if ci == 2 else 3))
                       for ci, (_, cw) in enumerate(CHUNKS[s])]
                if s == 0 and hk == 0:
                    # interleave g/v per j-tile: halves the x consumption
                    # rate so the first tile tracks the incoming x DMAs
                    for j in range(ND):
                        for w_t, pss in ((wg_t, pgs), (wv_t, pvs)):
                            lhsT = w_t[s][:, j, hs]
                            for ci, (cs, cw) in enumerate(CHUNKS[s]):
                                nc.tensor.matmul(
                                    pss[ci][:], lhsT=lhsT,
                                    rhs=x_sb[s][:, j, cs:cs + cw],
                                    start=(j == 0), stop=(j == ND - 1))
                else:
                    for w_t, pss in ((wg_t, pgs), (wv_t, pvs)):
                        for j in range(ND):
                            lhsT = w_t[s][:, j, hs]
                            for ci, (cs, cw) in enumerate(CHUNKS[s]):
                                nc.tensor.matmul(
                                    pss[ci][:], lhsT=lhsT,
                                    rhs=x_sb[s][:, j, cs:cs + cw],
                                    start=(j == 0), stop=(j == ND - 1))
                for ci, (cs, cw) in enumerate(CHUNKS[s]):
                    sg = act.tile([P, 512], FP, tag="sg")
                    nc.scalar.activation(sg[:, :cw], pgs[ci][:], AF.Sigmoid)
                    t1 = act.tile([P, 512], FP, tag="t1")
                    nc.vector.tensor_tensor(t1[:, :cw], pgs[ci][:],
                                            sg[:, :cw], op=OP.mult)
                    nc.vector.tensor_tensor(ht[s][hk][:, cs:cs + cw],
                                            t1[:, :cw], pvs[ci][:],
                                            op=OP.mult)

    # ---- phase B: yT[d, tok] = wo^T @ hT --------------------------------
    with ExitStack() as bctx:
        ps_y = bctx.enter_context(tc.tile_pool(name="psy", bufs=3, space="PSUM"))
        for s in range(NS):
            y_ap = tens[f"y{s}"].ap()
            for dt in range(ND):
                dsl = slice(dt * P, (dt + 1) * P)
                pys = [ps_y.tile([P, cw], FP, name=f"py{ci}", tag=f"py{ci}",
                                 bufs=(2 if ci == 2 else 3))
                       for ci, (_, cw) in enumerate(CHUNKS[s])]
                for hj in range(NHQ):
                    lhsT = wo_sb[s][:, hj, dsl]
                    for ci, (cs, cw) in enumerate(CHUNKS[s]):
                        nc.tensor.matmul(
                            pys[ci][:], lhsT=lhsT,
                            rhs=ht[s][hj][:, cs:cs + cw],
                            start=(hj == 0), stop=(hj == NHQ - 1))
                ysb = yst.tile([P, CAPS[s]], BFD, tag="y", name="ysb")
                for ci, (cs, cw) in enumerate(CHUNKS[s]):
                    nc.scalar.activation(ysb[:, cs:cs + cw], pys[ci][:],
                                         AF.Copy)
                eng = nc.gpsimd if (dt % 2 == 0) else nc.scalar
                eng.dma_start(out=y_ap[dsl, :], in_=ysb[:])


def _dedup_ldweights(nc):
    """Drop InstLdweights that reload the exact weights already resident
    in the PE array (identical AP/mode, no intervening clobber, no sync)."""
    removed = 0
    for blk in nc.main_func.blocks:
        last_key = None
        new = []
        for inst in blk.instructions:
            if isinstance(inst, mybir.InstLdweights):
                si = inst.sync_info
                clean = si is None or (not si.on_wait and not si.on_update)
                key = (
                    repr(inst.ins[0]), str(inst.perf_mode),
                    str(inst.is_transpose), str(inst.tile_position),
                    str(inst.tile_size),
                )
                if clean and key == last_key:
                    removed += 1
                    continue
                last_key = key
            elif isinstance(inst, mybir.InstMatmult):
                if inst.ldweights is None or inst.is_transpose:
                    last_key = None
            new.append(inst)
        blk.instructions[:] = new
    return removed


def _build():
    nc = bacc.Bacc("TRN2", target_bir_lowering=False, debug=False)
    tens = {}
    for s in range(NS):
        tens[f"x{s}"] = nc.dram_tensor(f"x{s}", [D, CAPS[s]], BFD,
                                       kind="ExternalInput")
        tens[f"wg{s}"] = nc.dram_tensor(f"wg{s}", [D, HQ], BFD,
                                        kind="ExternalInput")
        tens[f"wv{s}"] = nc.dram_tensor(f"wv{s}", [D, HQ], BFD,
                                        kind="ExternalInput")
        tens[f"wo{s}"] = nc.dram_tensor(f"wo{s}", [HQ, D], BFD,
                                        kind="ExternalInput")
        tens[f"y{s}"] = nc.dram_tensor(f"y{s}", [D, CAPS[s]], BFD,
                                       kind="ExternalOutput")
    with tile.TileContext(nc) as tc:
        with ExitStack() as ctx:
            _emit(nc, tc, ctx, tens)
    _dedup_ldweights(nc)
    nc.compile()
    return nc


_NC = None


def _get_nc():
    global _NC
    if _NC is None:
        _NC = _build()
    return _NC


def _route(xf, gate_w, expert_bias):
    """Host-side replica of the reference router."""
    logits = xf @ gate_w + expert_bias          # [N, E] fp32
    m = logits.max(axis=-1, keepdims=True)
    p = np.exp(logits - m)
    p /= p.sum(axis=-1, keepdims=True)
    # ties -> lower index first, matching jax.lax.top_k
    order = np.argsort(-p, axis=-1, kind="stable")[:, :TOP_K]
    rw = np.take_along_axis(p, order, -1)
    rw = rw / (rw.sum(-1, keepdims=True) + np.float32(1e-8))
    return order, rw


def _slot_inputs(xf, ids, cap, w_gate, w_value, w_out, expert, quarter):
    """Build one (expert, H-quarter) job's device inputs."""
    ids_p = np.zeros(cap, dtype=np.int64)
    ids_p[: len(ids)] = ids
    xt = np.ascontiguousarray(xf[ids_p].T.astype(BF))
    hsl = slice(quarter * HQ, (quarter + 1) * HQ)
    return {
        "x": xt,
        "wg": np.ascontiguousarray(w_gate[expert][:, hsl].astype(BF)),
        "wv": np.ascontiguousarray(w_value[expert][:, hsl].astype(BF)),
        "wo": np.ascontiguousarray(w_out[expert][hsl, :].astype(BF)),
    }


def kernel(x, gate_w, expert_bias, w_gate, w_value, w_out, _trace=False):
    x = np.asarray(x, dtype=np.float32)
    gate_w = np.asarray(gate_w, dtype=np.float32)
    expert_bias = np.asarray(expert_bias, dtype=np.float32)
    w_gate = np.asarray(w_gate, dtype=np.float32)
    w_value = np.asarray(w_value, dtype=np.float32)
    w_out = np.asarray(w_out, dtype=np.float32)

    xf = np.ascontiguousarray(x.reshape(N_TOK, D))
    order, rw = _route(xf, gate_w, expert_bias)
    idx = [np.flatnonzero((order == e).any(axis=-1)) for e in range(E)]

    # slot s <- (2s, 2s+1)-heaviest experts; their 4 quarters spread
    # across cores 4k..4k+3
    by_load = sorted(range(E), key=lambda e: -len(idx[e]))
    slot_exp = tuple(by_load[2 * s:2 * s + 2] for s in range(NS))
    n_rounds = max(
        max(1, math.ceil(len(idx[e]) / CAPS[s]))
        for s in range(NS) for e in slot_exp[s]
    )

    nc = _get_nc()
    out = np.zeros((N_TOK, D), dtype=np.float32)
    last = None
    for r in range(n_rounds):
        in_maps = []
        round_ids = [[], [], [], []]
        for c in range(E):
            m = {}
            for s in range(NS):
                e = slot_exp[s][c // 4]
                quarter = c % 4
                ids = idx[e][r * CAPS[s]:(r + 1) * CAPS[s]]
                round_ids[s].append(ids)
                job = _slot_inputs(xf, ids, CAPS[s],
                                   w_gate, w_value, w_out, e, quarter)
                for k in ("x", "wg", "wv", "wo"):
                    m[f"{k}{s}"] = job[k]
            in_maps.append(m)
        res = run_bass_kernel_spmd(
            nc, in_maps, core_ids=list(range(E)),
            trace=bool(_trace), trace_cores=list(range(E)) if _trace else None,
        )
        last = res
        for c in range(E):
            for s in range(NS):
                ids = round_ids[s][c]
                if len(ids):
                    e = slot_exp[s][c // 4]
                    sel = order[ids] == e
                    w_tok = np.where(sel[:, 0], rw[ids, 0], rw[ids, 1])
                    yT = res.results[c][f"y{s}"][:, : len(ids)]
                    out[ids] += w_tok[:, None].astype(np.float32) * \
                        yT.T.astype(np.float32)
    if _trace:
        kernel.last_results = last
    return out.reshape(B, T, D)


# revision 39
# speedup vs baseline: 1.0993x; 1.0993x over previous
"""MoE FFN (8 experts, top-2, SwiGLU) Trainium2 kernel — bf16 quarter-expert.

Sharding: each expert's hidden dim H=2048 is split into four quarters;
the 32 (expert, H-quarter) jobs are packed 4-per-core in 4 SPMD slots.
Slot s holds the quarters of the (2s, 2s+1)-heaviest-loaded experts
(per the host router), so the per-slot token capacity tracks the actual
expert loads (1092/1028/1020/1000 here vs 1152 for plain
expert-parallel SPMD). The four quarters of an expert produce partial
y sums the host adds during scatter.

The router (top-2 + combine weights) runs on host, exactly replicating
the reference. FFN matmuls are bf16 (measured PE rate: 1 cycle per
moving row, weight loads fully hidden). Phase A computes hT[h,tok]
h-major (g-pass then v-pass sharing PSUM tags generationally); phase B
computes yT[d,tok] d-major (wo stationary, exact token counts — no
padded-token rows); y is stored bf16 and combine-scaled on host.

Self-contained: shapes/sharding hardcoded for
x[2,2048,1024], 8 experts, d_expert=2048, top-2.
"""

import math
from contextlib import ExitStack

import ml_dtypes
import numpy as np

import concourse.mybir as mybir
import concourse.tile as tile
from concourse import bacc
from concourse.bass_utils import run_bass_kernel_spmd

# ---- problem constants --------------------------------------------------
B, T, D = 2, 2048, 1024
N_TOK = B * T          # 4096 tokens
E = 8                  # experts
H = 2048               # expert hidden dim
HQ = H // 4            # per-slot hidden quarter (512)
TOP_K = 2
P = 128
NS = 4                 # slots per core

CAPS = (1092, 1028, 1020, 1000)   # per-slot token capacity
ND = D // P            # 8  d-tiles
NHQ = HQ // P          # 4  h-tiles per slot

BFD = mybir.dt.bfloat16
FP = mybir.dt.float32
AF = mybir.ActivationFunctionType
OP = mybir.AluOpType
BF = ml_dtypes.bfloat16


def _chunks(cap):
    out, cs = [], 0
    while cs < cap:
        cw = min(512, cap - cs)
        out.append((cs, cw))
        cs += cw
    return out


CHUNKS = tuple(_chunks(c) for c in CAPS)


def _emit(nc, tc, ctx, tens):
    const = ctx.enter_context(tc.tile_pool(name="const", bufs=1))
    wsb = ctx.enter_context(tc.tile_pool(name="wsb", bufs=1))
    wgv = ctx.enter_context(tc.tile_pool(name="wgv", bufs=2))
    htp = ctx.enter_context(tc.tile_pool(name="htp", bufs=1))
    act = ctx.enter_context(tc.tile_pool(name="act", bufs=2))
    yst = ctx.enter_context(tc.tile_pool(name="yst", bufs=5))

    x_sb, x_ap, wg_ap, wv_ap, wo_ap = [], [], [], [], []
    wo_sb = []
    for s in range(NS):
        x_sb.append(const.tile([P, ND, CAPS[s]], BFD, name=f"x{s}"))
        x_ap.append(tens[f"x{s}"].ap().rearrange("(j p) c -> p j c", p=P))
        wg_ap.append(tens[f"wg{s}"].ap().rearrange("(j p) h -> p j h", p=P))
        wv_ap.append(tens[f"wv{s}"].ap().rearrange("(j p) h -> p j h", p=P))
        wo_sb.append(wsb.tile([P, NHQ, D], BFD, name=f"wo{s}"))
        wo_ap.append(tens[f"wo{s}"].ap().rearrange("(j p) d -> p j d", p=P))

    # wg/wv stream through a 2-deep pool (dead after their slot's A pass)
    wg_t, wv_t = [None] * NS, [None] * NS
    for s in range(NS):
        wg_t[s] = wgv.tile([P, ND, HQ], BFD, name=f"wg{s}", tag="wg")
        wv_t[s] = wgv.tile([P, ND, HQ], BFD, name=f"wv{s}", tag="wv")

    # head-latency-ordered loads on the sync queue, first-use order;
    # the scalar queue configures slot0's v-pass head load in parallel.
    nc.sync.dma_start(out=x_sb[0][:, 0:1, 0:512], in_=x_ap[0][:, 0:1, 0:512])
    nc.sync.dma_start(out=wg_t[0][:, :, 0:P], in_=wg_ap[0][:, :, 0:P])
    nc.sync.dma_start(out=x_sb[0][:, 0:1, 512:], in_=x_ap[0][:, 0:1, 512:])
    nc.sync.dma_start(out=x_sb[0][:, 1:2], in_=x_ap[0][:, 1:2])
    nc.sync.dma_start(out=x_sb[0][:, 2:4], in_=x_ap[0][:, 2:4])
    nc.sync.dma_start(out=x_sb[0][:, 4:6], in_=x_ap[0][:, 4:6])
    nc.sync.dma_start(out=x_sb[0][:, 6:8], in_=x_ap[0][:, 6:8])
    nc.scalar.dma_start(out=wv_t[0][:, :, 0:P], in_=wv_ap[0][:, :, 0:P])
    nc.sync.dma_start(out=wg_t[0][:, :, P:HQ], in_=wg_ap[0][:, :, P:HQ])
    nc.sync.dma_start(out=wv_t[0][:, :, P:HQ], in_=wv_ap[0][:, :, P:HQ])
    # slot1's wg/wv use the pool's second buffers (no wait); slots 2-3
    # reuse buffers freed by earlier slots' A passes, so their DMAs
    # carry waits — emit them LAST so they cannot head-of-line-block
    # the independent x/wo loads on the queue.
    nc.sync.dma_start(out=wg_t[1][:], in_=wg_ap[1])
    nc.sync.dma_start(out=wv_t[1][:], in_=wv_ap[1])
    for s in range(1, NS):
        nc.sync.dma_start(out=x_sb[s][:, 0:4], in_=x_ap[s][:, 0:4])
        nc.sync.dma_start(out=x_sb[s][:, 4:8], in_=x_ap[s][:, 4:8])
    for s in range(NS):
        nc.sync.dma_start(out=wo_sb[s][:], in_=wo_ap[s])
    for s in range(2, NS):
        nc.sync.dma_start(out=wg_t[s][:], in_=wg_ap[s])
        nc.sync.dma_start(out=wv_t[s][:], in_=wv_ap[s])

    ht = [[htp.tile([P, CAPS[s]], BFD, name=f"ht{s}_{k}") for k in range(NHQ)]
          for s in range(NS)]

    # PE p-state warm-up on a zeroed tile (no DMA dependency)
    warmz = const.tile([P, 2 * P], BFD)
    nc.vector.memset(warmz[:], 0.0)
    with ExitStack() as wctx:
        ps_w = wctx.enter_context(tc.tile_pool(name="psw", bufs=1, space="PSUM"))
        warm = ps_w.tile([P, 2 * P], FP, name="warm", tag="warm")
        for _ in range(24):
            nc.tensor.matmul(warm[:], lhsT=warmz[:, 0:P], rhs=warmz[:],
                             start=True, stop=True)

    # ---- phase A: hT[h, tok] = silu(x@wg)^T * (x@wv)^T ------------------
    with ExitStack() as actx:
        ps_a = actx.enter_context(tc.tile_pool(name="psa", bufs=3, space="PSUM"))
        for s in range(NS):
            for hk in range(NHQ):
                hs = slice(hk * P, (hk + 1) * P)
                pgs = [ps_a.tile([P, cw], FP, name=f"pg{ci}", tag=f"p{ci}",
                                 bufs=(2 if ci == 2 else 3))
                       for ci, (_, cw) in enumerate(CHUNKS[s])]
                pvs = [ps_a.tile([P, cw], FP, name=f"pv{ci}", tag=f"p{ci}",
                                 bufs=(2 if ci == 2 else 3))
                       for ci, (_, cw) in enumerate(CHUNKS[s])]
                if s == 0 and hk == 0:
                    # interleave g/v per j-tile: halves the x consumption
                    # rate so the first tile tracks the incoming x DMAs
                    for j in range(ND):
                        for w_t, pss in ((wg_t, pgs), (wv_t, pvs)):
                            lhsT = w_t[s][:, j, hs]
                            for ci, (cs, cw) in enumerate(CHUNKS[s]):
                                nc.tensor.matmul(
                                    pss[ci][:], lhsT=lhsT,
                                    rhs=x_sb[s][:, j, cs:cs + cw],
                                    start=(j == 0), stop=(j == ND - 1))
                else:
                    for w_t, pss in ((wg_t, pgs), (wv_t, pvs)):
                        for j in range(ND):
                            lhsT = w_t[s][:, j, hs]
                            for ci, (cs, cw) in enumerate(CHUNKS[s]):
                                nc.tensor.matmul(
                                    pss[ci][:], lhsT=lhsT,
                                    rhs=x_sb[s][:, j, cs:cs + cw],
                                    start=(j == 0), stop=(j == ND - 1))
                for ci, (cs, cw) in enumerate(CHUNKS[s]):
                    sg = act.tile([P, 512], FP, tag="sg")
                    nc.scalar.activation(sg[:, :cw], pgs[ci][:], AF.Sigmoid)
                    t1 = act.tile([P, 512], FP, tag="t1")
                    nc.vector.tensor_tensor(t1[:, :cw], pgs[ci][:],
                                            sg[:, :cw], op=OP.mult)
                    nc.vector.tensor_tensor(ht[s][hk][:, cs:cs + cw],
                                            t1[:, :cw], pvs[ci][:],
                                            op=OP.mult)

    # ---- phase B: yT[d, tok] = wo^T @ hT --------------------------------
    with ExitStack() as bctx:
        ps_y = bctx.enter_context(tc.tile_pool(name="psy", bufs=3, space="PSUM"))
        for s in range(NS):
            y_ap = tens[f"y{s}"].ap()
            for dt in range(ND):
                dsl = slice(dt * P, (dt + 1) * P)
                pys = [ps_y.tile([P, cw], FP, name=f"py{ci}", tag=f"py{ci}",
                                 bufs=(2 if ci == 2 else 3))
                       for ci, (_, cw) in enumerate(CHUNKS[s])]
                for hj in range(NHQ):
                    lhsT = wo_sb[s][:, hj, dsl]
                    for ci, (cs, cw) in enumerate(CHUNKS[s]):
                        nc.tensor.matmul(
                            pys[ci][:], lhsT=lhsT,
                            rhs=ht[s][hj][:, cs:cs + cw],
                            start=(hj == 0), stop=(hj == NHQ - 1))
                ysb = yst.tile([P, CAPS[s]], BFD, tag="y", name="ysb")
                for ci, (cs, cw) in enumerate(CHUNKS[s]):
                    nc.scalar.activation(ysb[:, cs:cs + cw], pys[ci][:],
                                         AF.Copy)
                eng = nc.gpsimd if (dt % 2 == 0) else nc.scalar
                eng.dma_start(out=y_ap[dsl, :], in_=ysb[:])


def _dedup_ldweights(nc):
    """Drop InstLdweights that reload the exact weights already resident
    in the PE array (identical AP/mode, no intervening clobber, no sync)."""
    removed = 0
    for blk in nc.main_func.blocks:
        last_key = None
        new = []
        for inst in blk.instructions:
            if isinstance(inst, mybir.InstLdweights):
                si = inst.sync_info
                clean = si is None or (not si.on_wait and not si.on_update)
                key = (
                    repr(inst.ins[0]), str(inst.perf_mode),
                    str(inst.is_transpose), str(inst.tile_position),
                    str(inst.tile_size),
                )
                if clean and key == last_key:
                    removed += 1
                    continue
                last_key = key
            elif isinstance(inst, mybir.InstMatmult):
                if inst.ldweights is None or inst.is_transpose:
                    last_key = None
            new.append(inst)
        blk.instructions[:] = new
    return removed


def _build():
    nc = bacc.Bacc("TRN2", target_bir_lowering=False, debug=False)
    tens = {}
    for s in range(NS):
        tens[f"x{s}"] = nc.dram_tensor(f"x{s}", [D, CAPS[s]], BFD,
                                       kind="ExternalInput")
        tens[f"wg{s}"] = nc.dram_tensor(f"wg{s}", [D, HQ], BFD,
                                        kind="ExternalInput")
        tens[f"wv{s}"] = nc.dram_tensor(f"wv{s}", [D, HQ], BFD,
                                        kind="ExternalInput")
        tens[f"wo{s}"] = nc.dram_tensor(f"wo{s}", [HQ, D], BFD,
                                        kind="ExternalInput")
        tens[f"y{s}"] = nc.dram_tensor(f"y{s}", [D, CAPS[s]], BFD,
                                       kind="ExternalOutput")
    with tile.TileContext(nc) as tc:
        with ExitStack() as ctx:
            _emit(nc, tc, ctx, tens)
    _dedup_ldweights(nc)
    nc.compile()
    return nc


_NC = None


def _get_nc():
    global _NC
    if _NC is None:
        _NC = _build()
    return _NC


def _route(xf, gate_w, expert_bias):
    """Host-side replica of the reference router."""
    logits = xf @ gate_w + expert_bias          # [N, E] fp32
    m = logits.max(axis=-1, keepdims=True)
    p = np.exp(logits - m)
    p /= p.sum(axis=-1, keepdims=True)
    # ties -> lower index first, matching jax.lax.top_k
    order = np.argsort(-p, axis=-1, kind="stable")[:, :TOP_K]
    rw = np.take_along_axis(p, order, -1)
    rw = rw / (rw.sum(-1, keepdims=True) + np.float32(1e-8))
    return order, rw


def _slot_inputs(xf, ids, cap, w_gate, w_value, w_out, expert, quarter):
    """Build one (expert, H-quarter) job's device inputs."""
    ids_p = np.zeros(cap, dtype=np.int64)
    ids_p[: len(ids)] = ids
    xt = np.ascontiguousarray(xf[ids_p].T.astype(BF))
    hsl = slice(quarter * HQ, (quarter + 1) * HQ)
    return {
        "x": xt,
        "wg": np.ascontiguousarray(w_gate[expert][:, hsl].astype(BF)),
        "wv": np.ascontiguousarray(w_value[expert][:, hsl].astype(BF)),
        "wo": np.ascontiguousarray(w_out[expert][hsl, :].astype(BF)),
    }


def kernel(x, gate_w, expert_bias, w_gate, w_value, w_out, _trace=False):
    x = np.asarray(x, dtype=np.float32)
    gate_w = np.asarray(gate_w, dtype=np.float32)
    expert_bias = np.asarray(expert_bias, dtype=np.float32)
    w_gate = np.asarray(w_gate, dtype=np.float32)
    w_value = np.asarray(w_value, dtype=np.float32)
    w_out = np.asarray(w_out, dtype=np.float32)

    xf = np.ascontiguousarray(x.reshape(N_TOK, D))
    order, rw = _route(xf, gate_w, expert_bias)
    idx = [np.flatnonzero((order == e).any(axis=-1)) for e in range(E)]

    # slot s <- (2s, 2s+1)-heaviest experts; their 4 quarters spread
    # across cores 4k..4k+3
    by_load = sorted(range(E), key=lambda e: -len(idx[e]))
    slot_exp = tuple(by_load[2 * s:2 * s + 2] for s in range(NS))
    n_rounds = max(
        max(1, math.ceil(len(idx[e]) / CAPS[s]))
        for s in range(NS) for e in slot_exp[s]
    )

    nc = _get_nc()
    out = np.zeros((N_TOK, D), dtype=np.float32)
    last = None
    for r in range(n_rounds):
        in_maps = []
        round_ids = [[], [], [], []]
        for c in range(E):
            m = {}
            for s in range(NS):
                e = slot_exp[s][c // 4]
                quarter = c % 4
                ids = idx[e][r * CAPS[s]:(r + 1) * CAPS[s]]
                round_ids[s].append(ids)
                job = _slot_inputs(xf, ids, CAPS[s],
                                   w_gate, w_value, w_out, e, quarter)
                for k in ("x", "wg", "wv", "wo"):
                    m[f"{k}{s}"] = job[k]
            in_maps.append(m)
        res = run_bass_kernel_spmd(
            nc, in_maps, core_ids=list(range(E)),
            trace=bool(_trace), trace_cores=list(range(E)) if _trace else None,
        )
        last = res
        for c in range(E):
            for s in range(NS):
                ids = round_ids[s][c]
                if len(ids):
                    e = slot_exp[s][c // 4]
                    sel = order[ids] == e
                    w_tok = np.where(sel[:, 0], rw[ids, 0], rw[ids, 1])
                    yT = res.results[c][f"y{s}"][:, : len(ids)]
                    out[ids] += w_tok[:, None].astype(np.float32) * \
                        yT.T.astype(np.float32)
    if _trace:
        kernel.last_results = last
    return out.reshape(B, T, D)


# revision 40
# speedup vs baseline: 1.1110x; 1.0107x over previous
"""MoE FFN (8 experts, top-2, SwiGLU) Trainium2 kernel — bf16 quarter-expert.

Sharding: each expert's hidden dim H=2048 is split into four quarters;
the 32 (expert, H-quarter) jobs are packed 4-per-core in 4 SPMD slots.
Slot s holds the quarters of the (2s, 2s+1)-heaviest-loaded experts
(per the host router), so the per-slot token capacity tracks the actual
expert loads (1092/1028/1020/1000 here vs 1152 for plain
expert-parallel SPMD). The four quarters of an expert produce partial
y sums the host adds during scatter.

The router (top-2 + combine weights) runs on host, exactly replicating
the reference. FFN matmuls are bf16 (measured PE rate: 1 cycle per
moving row, weight loads fully hidden). Phase A computes hT[h,tok]
h-major (g-pass then v-pass sharing PSUM tags generationally); phase B
computes yT[d,tok] d-major (wo stationary, exact token counts — no
padded-token rows); y is stored bf16 and combine-scaled on host.

Self-contained: shapes/sharding hardcoded for
x[2,2048,1024], 8 experts, d_expert=2048, top-2.
"""

import math
from contextlib import ExitStack

import ml_dtypes
import numpy as np

import concourse.mybir as mybir
import concourse.tile as tile
from concourse import bacc
from concourse.bass_utils import run_bass_kernel_spmd

# ---- problem constants --------------------------------------------------
B, T, D = 2, 2048, 1024
N_TOK = B * T          # 4096 tokens
E = 8                  # experts
H = 2048               # expert hidden dim
HQ = H // 4            # per-slot hidden quarter (512)
TOP_K = 2
P = 128
NS = 4                 # slots per core

CAPS = (1092, 1028, 1020, 1000)   # per-slot token capacity
ND = D // P            # 8  d-tiles
NHQ = HQ // P          # 4  h-tiles per slot

BFD = mybir.dt.bfloat16
FP = mybir.dt.float32
AF = mybir.ActivationFunctionType
OP = mybir.AluOpType
BF = ml_dtypes.bfloat16


def _chunks(cap):
    out, cs = [], 0
    while cs < cap:
        cw = min(512, cap - cs)
        out.append((cs, cw))
        cs += cw
    return out


CHUNKS = tuple(_chunks(c) for c in CAPS)


def _emit(nc, tc, ctx, tens):
    const = ctx.enter_context(tc.tile_pool(name="const", bufs=1))
    wsb = ctx.enter_context(tc.tile_pool(name="wsb", bufs=1))
    wgv = ctx.enter_context(tc.tile_pool(name="wgv", bufs=2))
    htp = ctx.enter_context(tc.tile_pool(name="htp", bufs=1))
    act = ctx.enter_context(tc.tile_pool(name="act", bufs=2))
    yst = ctx.enter_context(tc.tile_pool(name="yst", bufs=5))

    x_sb, x_ap, wg_ap, wv_ap, wo_ap = [], [], [], [], []
    wo_sb = []
    for s in range(NS):
        x_sb.append(const.tile([P, ND, CAPS[s]], BFD, name=f"x{s}"))
        x_ap.append(tens[f"x{s}"].ap().rearrange("(j p) c -> p j c", p=P))
        wg_ap.append(tens[f"wg{s}"].ap().rearrange("(j p) h -> p j h", p=P))
        wv_ap.append(tens[f"wv{s}"].ap().rearrange("(j p) h -> p j h", p=P))
        wo_sb.append(wsb.tile([P, NHQ, D], BFD, name=f"wo{s}"))
        wo_ap.append(tens[f"wo{s}"].ap().rearrange("(j p) d -> p j d", p=P))

    # wg/wv stream through a 2-deep pool (dead after their slot's A pass)
    wg_t, wv_t = [None] * NS, [None] * NS
    for s in range(NS):
        wg_t[s] = wgv.tile([P, ND, HQ], BFD, name=f"wg{s}", tag="wg")
        wv_t[s] = wgv.tile([P, ND, HQ], BFD, name=f"wv{s}", tag="wv")

    # head-latency-ordered loads on the sync queue, first-use order;
    # the scalar queue configures slot0's v-pass head load in parallel.
    nc.sync.dma_start(out=x_sb[0][:, 0:1, 0:512], in_=x_ap[0][:, 0:1, 0:512])
    nc.sync.dma_start(out=wg_t[0][:, :, 0:P], in_=wg_ap[0][:, :, 0:P])
    nc.sync.dma_start(out=x_sb[0][:, 0:1, 512:], in_=x_ap[0][:, 0:1, 512:])
    nc.sync.dma_start(out=x_sb[0][:, 1:2], in_=x_ap[0][:, 1:2])
    nc.sync.dma_start(out=x_sb[0][:, 2:4], in_=x_ap[0][:, 2:4])
    nc.sync.dma_start(out=x_sb[0][:, 4:6], in_=x_ap[0][:, 4:6])
    nc.sync.dma_start(out=x_sb[0][:, 6:8], in_=x_ap[0][:, 6:8])
    nc.scalar.dma_start(out=wv_t[0][:, :, 0:P], in_=wv_ap[0][:, :, 0:P])
    nc.sync.dma_start(out=wg_t[0][:, :, P:HQ], in_=wg_ap[0][:, :, P:HQ])
    nc.sync.dma_start(out=wv_t[0][:, :, P:HQ], in_=wv_ap[0][:, :, P:HQ])
    # slot1's wg/wv use the pool's second buffers (no wait); slots 2-3
    # reuse buffers freed by earlier slots' A passes, so their DMAs
    # carry waits — emit them LAST so they cannot head-of-line-block
    # the independent x/wo loads on the queue.
    nc.sync.dma_start(out=wg_t[1][:], in_=wg_ap[1])
    nc.sync.dma_start(out=wv_t[1][:], in_=wv_ap[1])
    for s in range(1, NS):
        nc.sync.dma_start(out=x_sb[s][:, 0:4], in_=x_ap[s][:, 0:4])
        nc.sync.dma_start(out=x_sb[s][:, 4:8], in_=x_ap[s][:, 4:8])
    for s in range(NS):
        nc.sync.dma_start(out=wo_sb[s][:], in_=wo_ap[s])
    for s in range(2, NS):
        nc.sync.dma_start(out=wg_t[s][:], in_=wg_ap[s])
        nc.sync.dma_start(out=wv_t[s][:], in_=wv_ap[s])

    ht = [[htp.tile([P, CAPS[s]], BFD, name=f"ht{s}_{k}") for k in range(NHQ)]
          for s in range(NS)]

    # PE p-state warm-up on a zeroed tile (no DMA dependency)
    warmz = const.tile([P, 2 * P], BFD)
    nc.vector.memset(warmz[:], 0.0)
    with ExitStack() as wctx:
        ps_w = wctx.enter_context(tc.tile_pool(name="psw", bufs=1, space="PSUM"))
        warm = ps_w.tile([P, 2 * P], FP, name="warm", tag="warm")
        for _ in range(24):
            nc.tensor.matmul(warm[:], lhsT=warmz[:, 0:P], rhs=warmz[:],
                             start=True, stop=True)

    # ---- phase A: hT[h, tok] = silu(x@wg)^T * (x@wv)^T ------------------
    with ExitStack() as actx:
        ps_a = actx.enter_context(tc.tile_pool(name="psa", bufs=3, space="PSUM"))
        for s in range(NS):
            for hk in range(NHQ):
                hs = slice(hk * P, (hk + 1) * P)
                pgs = [ps_a.tile([P, cw], FP, name=f"pg{ci}", tag=f"p{ci}",
                                 bufs=(2 if ci == 2 else 3))
                       for ci, (_, cw) in enumerate(CHUNKS[s])]
                pvs = [ps_a.tile([P, cw], FP, name=f"pv{ci}", tag=f"p{ci}",
                                 bufs=(2 if ci == 2 else 3))
                       for ci, (_, cw) in enumerate(CHUNKS[s])]
                if s == 0 and hk == 0:
                    # interleave g/v per j-tile: halves the x consumption
                    # rate so the first tile tracks the incoming x DMAs
                    for j in range(ND):
                        for w_t, pss in ((wg_t, pgs), (wv_t, pvs)):
                            lhsT = w_t[s][:, j, hs]
                            for ci, (cs, cw) in enumerate(CHUNKS[s]):
                                nc.tensor.matmul(
                                    pss[ci][:], lhsT=lhsT,
                                    rhs=x_sb[s][:, j, cs:cs + cw],
                                    start=(j == 0), stop=(j == ND - 1))
                else:
                    for w_t, pss in ((wg_t, pgs), (wv_t, pvs)):
                        for j in range(ND):
                            lhsT = w_t[s][:, j, hs]
                            for ci, (cs, cw) in enumerate(CHUNKS[s]):
                                nc.tensor.matmul(
                                    pss[ci][:], lhsT=lhsT,
                                    rhs=x_sb[s][:, j, cs:cs + cw],
                                    start=(j == 0), stop=(j == ND - 1))
                for ci, (cs, cw) in enumerate(CHUNKS[s]):
                    sg = act.tile([P, 512], FP, tag="sg")
                    nc.scalar.activation(sg[:, :cw], pgs[ci][:], AF.Sigmoid)
                    t1 = act.tile([P, 512], FP, tag="t1")
                    nc.vector.tensor_tensor(t1[:, :cw], pgs[ci][:],
                                            sg[:, :cw], op=OP.mult)
                    nc.vector.tensor_tensor(ht[s][hk][:, cs:cs + cw],
                                            t1[:, :cw], pvs[ci][:],
                                            op=OP.mult)

    # ---- phase B: yT[d, tok] = wo^T @ hT --------------------------------
    # adjacent dt rows are contiguous in yT, so stores batch in pairs:
    # 16 DMAs instead of 32 halves the per-store SWDGE sequencing that
    # otherwise trails past the last matmul.
    with ExitStack() as bctx:
        ps_y = bctx.enter_context(tc.tile_pool(name="psy", bufs=3, space="PSUM"))
        for s in range(NS):
            y2_ap = tens[f"y{s}"].ap().rearrange("(t i p) c -> p t i c",
                                                 i=2, p=P)
            for dt in range(ND):
                dsl = slice(dt * P, (dt + 1) * P)
                pys = [ps_y.tile([P, cw], FP, name=f"py{ci}", tag=f"py{ci}",
                                 bufs=(2 if ci == 2 else 3))
                       for ci, (_, cw) in enumerate(CHUNKS[s])]
                for hj in range(NHQ):
                    lhsT = wo_sb[s][:, hj, dsl]
                    for ci, (cs, cw) in enumerate(CHUNKS[s]):
                        nc.tensor.matmul(
                            pys[ci][:], lhsT=lhsT,
                            rhs=ht[s][hj][:, cs:cs + cw],
                            start=(hj == 0), stop=(hj == NHQ - 1))
                if dt % 2 == 0:
                    ysb = yst.tile([P, 2, CAPS[s]], BFD, tag="y", name="ysb")
                for ci, (cs, cw) in enumerate(CHUNKS[s]):
                    nc.scalar.activation(ysb[:, dt % 2, cs:cs + cw],
                                         pys[ci][:], AF.Copy)
                if dt % 2 == 1:
                    eng = nc.gpsimd if (dt % 4 == 1) else nc.scalar
                    eng.dma_start(out=y2_ap[:, dt // 2], in_=ysb[:])


def _dedup_ldweights(nc):
    """Drop InstLdweights that reload the exact weights already resident
    in the PE array (identical AP/mode, no intervening clobber, no sync)."""
    removed = 0
    for blk in nc.main_func.blocks:
        last_key = None
        new = []
        for inst in blk.instructions:
            if isinstance(inst, mybir.InstLdweights):
                si = inst.sync_info
                clean = si is None or (not si.on_wait and not si.on_update)
                key = (
                    repr(inst.ins[0]), str(inst.perf_mode),
                    str(inst.is_transpose), str(inst.tile_position),
                    str(inst.tile_size),
                )
                if clean and key == last_key:
                    removed += 1
                    continue
                last_key = key
            elif isinstance(inst, mybir.InstMatmult):
                if inst.ldweights is None or inst.is_transpose:
                    last_key = None
            new.append(inst)
        blk.instructions[:] = new
    return removed


def _build():
    nc = bacc.Bacc("TRN2", target_bir_lowering=False, debug=False)
    tens = {}
    for s in range(NS):
        tens[f"x{s}"] = nc.dram_tensor(f"x{s}", [D, CAPS[s]], BFD,
                                       kind="ExternalInput")
        tens[f"wg{s}"] = nc.dram_tensor(f"wg{s}", [D, HQ], BFD,
                                        kind="ExternalInput")
        tens[f"wv{s}"] = nc.dram_tensor(f"wv{s}", [D, HQ], BFD,
                                        kind="ExternalInput")
        tens[f"wo{s}"] = nc.dram_tensor(f"wo{s}", [HQ, D], BFD,
                                        kind="ExternalInput")
        tens[f"y{s}"] = nc.dram_tensor(f"y{s}", [D, CAPS[s]], BFD,
                                       kind="ExternalOutput")
    with tile.TileContext(nc) as tc:
        with ExitStack() as ctx:
            _emit(nc, tc, ctx, tens)
    _dedup_ldweights(nc)
    nc.compile()
    return nc


_NC = None


def _get_nc():
    global _NC
    if _NC is None:
        _NC = _build()
    return _NC


def _route(xf, gate_w, expert_bias):
    """Host-side replica of the reference router."""
    logits = xf @ gate_w + expert_bias          # [N, E] fp32
    m = logits.max(axis=-1, keepdims=True)
    p = np.exp(logits - m)
    p /= p.sum(axis=-1, keepdims=True)
    # ties -> lower index first, matching jax.lax.top_k
    order = np.argsort(-p, axis=-1, kind="stable")[:, :TOP_K]
    rw = np.take_along_axis(p, order, -1)
    rw = rw / (rw.sum(-1, keepdims=True) + np.float32(1e-8))
    return order, rw


def _slot_inputs(xf, ids, cap, w_gate, w_value, w_out, expert, quarter):
    """Build one (expert, H-quarter) job's device inputs."""
    ids_p = np.zeros(cap, dtype=np.int64)
    ids_p[: len(ids)] = ids
    xt = np.ascontiguousarray(xf[ids_p].T.astype(BF))
    hsl = slice(quarter * HQ, (quarter + 1) * HQ)
    return {
        "x": xt,
        "wg": np.ascontiguousarray(w_gate[expert][:, hsl].astype(BF)),
        "wv": np.ascontiguousarray(w_value[expert][:, hsl].astype(BF)),
        "wo": np.ascontiguousarray(w_out[expert][hsl, :].astype(BF)),
    }


def kernel(x, gate_w, expert_bias, w_gate, w_value, w_out, _trace=False):
    x = np.asarray(x, dtype=np.float32)
    gate_w = np.asarray(gate_w, dtype=np.float32)
    expert_bias = np.asarray(expert_bias, dtype=np.float32)
    w_gate = np.asarray(w_gate, dtype=np.float32)
    w_value = np.asarray(w_value, dtype=np.float32)
    w_out = np.asarray(w_out, dtype=np.float32)

    xf = np.ascontiguousarray(x.reshape(N_TOK, D))
    order, rw = _route(xf, gate_w, expert_bias)
    idx = [np.flatnonzero((order == e).any(axis=-1)) for e in range(E)]

    # slot s <- (2s, 2s+1)-heaviest experts; their 4 quarters spread
    # across cores 4k..4k+3
    by_load = sorted(range(E), key=lambda e: -len(idx[e]))
    slot_exp = tuple(by_load[2 * s:2 * s + 2] for s in range(NS))
    n_rounds = max(
        max(1, math.ceil(len(idx[e]) / CAPS[s]))
        for s in range(NS) for e in slot_exp[s]
    )

    nc = _get_nc()
    out = np.zeros((N_TOK, D), dtype=np.float32)
    last = None
    for r in range(n_rounds):
        in_maps = []
        round_ids = [[], [], [], []]
        for c in range(E):
            m = {}
            for s in range(NS):
                e = slot_exp[s][c // 4]
                quarter = c % 4
                ids = idx[e][r * CAPS[s]:(r + 1) * CAPS[s]]
                round_ids[s].append(ids)
                job = _slot_inputs(xf, ids, CAPS[s],
                                   w_gate, w_value, w_out, e, quarter)
                for k in ("x", "wg", "wv", "wo"):
                    m[f"{k}{s}"] = job[k]
            in_maps.append(m)
        res = run_bass_kernel_spmd(
            nc, in_maps, core_ids=list(range(E)),
            trace=bool(_trace), trace_cores=list(range(E)) if _trace else None,
        )
        last = res
        for c in range(E):
            for s in range(NS):
                ids = round_ids[s][c]
                if len(ids):
                    e = slot_exp[s][c // 4]
                    sel = order[ids] == e
                    w_tok = np.where(sel[:, 0], rw[ids, 0], rw[ids, 1])
                    yT = res.results[c][f"y{s}"][:, : len(ids)]
                    out[ids] += w_tok[:, None].astype(np.float32) * \
                        yT.T.astype(np.float32)
    if _trace:
        kernel.last_results = last
    return out.reshape(B, T, D)
